# revision 6
# baseline (speedup 1.0000x reference)
"""Trainium2 Bass kernel for nn_HLALayer (higher-order linear attention).

Math: the reference scan
    k_C = k_t @ C;  G += k_t k_t^T C;  S += k_t k_t^T;  C += q_t v_t^T
    o_t = q_t @ (S C - G)
admits a chunked closed form (chunk L):
    o_chunk = Q @ D0 + tril(Q S0 Q^T + A tril(A)^T) @ V,   A = Q K^T
with per-chunk state updates
    dS = K^T K;  dC = Q^T V;  G1 = G0 + dS C0 + K^T(stril(K Q^T) V)
    S1 = S0+dS; C1 = C0+dC; D1 = S1 C1 - G1

v2: bf16 operands everywhere (PSUM accumulation stays f32) -> FWL weight
loads, halved DMA; host passes x pre-transposed so no PE transposes of x;
trimmed triangular matmuls (right-half-only N1/M1/AT1); head-pair packing
of the 64-wide matmuls onto disjoint PE row/col groups.

Sharding: 8 cores = (batch b in {0,1}) x (head-group g in {0..3}, 4 heads
each).  Each core projects x[b] with its weight column/row slices, runs the
chunked scan for its 4 heads, applies its W_o row-slice -> partial [T, D]
bf16 output; the host sums the 4 partials per batch in f32.
"""

import numpy as np
import sys

sys.path.insert(0, "/opt/trn_rl_repo")

import ml_dtypes
import concourse.bacc as bacc
import concourse.mybir as mybir
from concourse.bass_utils import run_bass_kernel_spmd
from concourse.tile import TileContext

F32 = mybir.dt.float32
BF16 = mybir.dt.bfloat16
BF = ml_dtypes.bfloat16

D = 1024          # model dim
DL = 256          # per-core projection width (4 heads x 64)
DK = 64           # head dim
L = 256           # chunk length
NHL = 4           # local heads per core


def build(T=2048):
    NCH = T // L
    nc = bacc.Bacc("TRN2", target_bir_lowering=False)

    xt_in = nc.declare_dram_parameter("xt", [D, T], BF16, isOutput=False)
    wq_in = nc.declare_dram_parameter("wq", [D, DL], BF16, isOutput=False)
    wk_in = nc.declare_dram_parameter("wk", [D, DL], BF16, isOutput=False)
    wv_in = nc.declare_dram_parameter("wv", [D, DL], BF16, isOutput=False)
    wo_in = nc.declare_dram_parameter("wo", [DL, D], BF16, isOutput=False)
    id_in = nc.declare_dram_parameter("ident", [128, 128], BF16, isOutput=False)
    mtr_in = nc.declare_dram_parameter("mtr", [128, 128], F32, isOutput=False)  # triu
    mst_in = nc.declare_dram_parameter("mst", [128, 128], F32, isOutput=False)  # striu
    mt0_in = nc.declare_dram_parameter("mt0", [128, 256], F32, isOutput=False)  # [triu|1]
    out_d = nc.declare_dram_parameter("out", [T, D], BF16, isOutput=True)

    ncp = 0  # copy-engine round robin counter

    with TileContext(nc) as tc:
        with tc.tile_pool(name="const", bufs=1) as cpool, \
             tc.tile_pool(name="work", bufs=2) as work, \
             tc.tile_pool(name="spool", bufs=2) as spool, \
             tc.tile_pool(name="pp", bufs=2, space="PSUM") as pps:

            def cp(out_ap, in_ap):
                """plain copy/cast, alternating DVE / ACT to balance load"""
                nonlocal ncp
                ncp += 1
                if ncp % 2 == 0:
                    nc.vector.tensor_copy(out_ap, in_ap)
                else:
                    nc.scalar.copy(out_ap, in_ap)

            # ---- constants / weights (gpsimd=SWDGE queue, keeps HWDGE free) ----
            ident = cpool.tile([128, 128], BF16)
            nc.gpsimd.dma_start(out=ident[:], in_=id_in[:])
            mtr = cpool.tile([128, 128], F32)
            nc.gpsimd.dma_start(out=mtr[:], in_=mtr_in[:])
            mst = cpool.tile([128, 128], F32)
            nc.gpsimd.dma_start(out=mst[:], in_=mst_in[:])
            mt0 = cpool.tile([128, 256], F32)
            nc.gpsimd.dma_start(out=mt0[:], in_=mt0_in[:])

            wq_sb, wk_sb, wv_sb = [], [], []
            for j in range(8):
                wqt = cpool.tile([128, DL], BF16, name=f"wq{j}")
                nc.gpsimd.dma_start(out=wqt[:], in_=wq_in[128 * j:128 * (j + 1), :])
                wq_sb.append(wqt)
                wkt = cpool.tile([128, DL], BF16, name=f"wk{j}")
                nc.gpsimd.dma_start(out=wkt[:], in_=wk_in[128 * j:128 * (j + 1), :])
                wk_sb.append(wkt)
                wvt = cpool.tile([128, DL], BF16, name=f"wv{j}")
                nc.gpsimd.dma_start(out=wvt[:], in_=wv_in[128 * j:128 * (j + 1), :])
                wv_sb.append(wvt)
            wo_sb = []
            for m in range(2):
                wot = cpool.tile([128, D], BF16, name=f"wo{m}")
                nc.gpsimd.dma_start(out=wot[:], in_=wo_in[128 * m:128 * (m + 1), :])
                wo_sb.append(wot)

            # per-pair states (h0 at partitions 0:64, h1 at 64:128), bf16
            S = [None] * 2
            C = [None] * 2
            G = [None] * 2
            Dst = [None] * 2

            for c in range(NCH):
                t0 = L * c
                # ---------- phase 1: load x^T tiles, project ----------
                xt = []
                for j in range(8):
                    xtj = work.tile([128, L], BF16, tag=f"xt{j}", bufs=2,
                                    name=f"xt{j}_{c}")
                    nc.sync.dma_start(out=xtj[:],
                                      in_=xt_in[128 * j:128 * (j + 1), t0:t0 + L])
                    xt.append(xtj)

                qt, kt = [], []
                for m in range(2):
                    psq = pps.tile([128, L], F32, tag="pb", bufs=5, name=f"psq{m}_{c}")
                    for j in range(8):
                        nc.tensor.matmul(psq[:], wq_sb[j][:, 128 * m:128 * (m + 1)],
                                         xt[j][:], start=(j == 0), stop=(j == 7))
                    qtm = work.tile([128, L], BF16, tag=f"qt{m}", bufs=2,
                                    name=f"qt{m}_{c}")
                    cp(qtm[:], psq[:])
                    qt.append(qtm)
                    psk = pps.tile([128, L], F32, tag="pb", bufs=5, name=f"psk{m}_{c}")
                    for j in range(8):
                        nc.tensor.matmul(psk[:], wk_sb[j][:, 128 * m:128 * (m + 1)],
                                         xt[j][:], start=(j == 0), stop=(j == 7))
                    ktm = work.tile([128, L], BF16, tag=f"kt{m}", bufs=2,
                                    name=f"kt{m}_{c}")
                    cp(ktm[:], psk[:])
                    kt.append(ktm)
                vt = []
                for bb in range(2):
                    psv = pps.tile([128, DL], F32, tag="pb", bufs=5, name=f"psv{bb}_{c}")
                    for j in range(8):
                        nc.tensor.matmul(psv[:], xt[j][:, 128 * bb:128 * (bb + 1)],
                                         wv_sb[j][:], start=(j == 0), stop=(j == 7))
                    vtb = work.tile([128, DL], BF16, tag=f"vt{bb}", bufs=2,
                                    name=f"vt{bb}_{c}")
                    cp(vtb[:], psv[:])
                    vt.append(vtb)

                # per-chunk output tiles (oT layout [dv, t]; m=0: heads 0,1)
                ot = [work.tile([128, L], BF16, tag=f"ot{m}", bufs=2,
                                name=f"ot{m}_{c}") for m in range(2)]

                # ---------- phase 2: chunked scan, head pairs ----------
                for p in range(2):
                    heads = (2 * p, 2 * p + 1)
                    QT, KT, Vbs, idb = {}, {}, {}, {}
                    for h in heads:
                        m, po = h >> 1, 64 * (h & 1)
                        QT[h] = qt[m][po:po + 64, :]
                        KT[h] = kt[m][po:po + 64, :]
                        Vbs[h] = [vt[bb][:, 64 * h:64 * h + 64] for bb in range(2)]
                        idb[h] = ident[po:po + 64, po:po + 64]

                    # NM products: psNM = [N0 (256) | N1 right (128)], psM likewise
                    # (MM emission alternates heads so consecutive PE instrs hit
                    # disjoint row groups -> LDWEIGHTS overlaps the other head's MM)
                    psNM, psM = {}, {}
                    for h in heads:
                        psNM[h] = pps.tile([128, 384], F32, tag="pb", bufs=5,
                                           name=f"psNM_{c}_{h}")
                        psM[h] = pps.tile([128, 384], F32, tag="pb", bufs=5,
                                          name=f"psM_{c}_{h}")
                    for h in heads:
                        nc.tensor.matmul(psNM[h][:, 0:256], KT[h][:, 0:128], QT[h],
                                         start=True, stop=True)
                    for h in heads:
                        nc.tensor.matmul(psNM[h][:, 256:384], KT[h][:, 128:256],
                                         QT[h][:, 128:256], start=True, stop=True)
                    for h in heads:
                        nc.tensor.matmul(psM[h][:, 0:256], QT[h][:, 0:128], KT[h],
                                         start=True, stop=True)
                    for h in heads:
                        nc.tensor.matmul(psM[h][:, 256:384], QT[h][:, 128:256],
                                         KT[h][:, 128:256], start=True, stop=True)

                    # natural q/k via PE transpose (dk 64 -> partitions)
                    qn, kn = {}, {}
                    psqn, pskn = {}, {}
                    for h in heads:
                        qn[h], kn[h] = [], []
                        psqn[h], pskn[h] = [], []
                        for bb in range(2):
                            psqn[h].append(pps.tile([128, 64], BF16, tag="ps", bufs=3,
                                                    name=f"psqn{bb}_{c}_{h}"))
                            pskn[h].append(pps.tile([128, 64], BF16, tag="ps", bufs=3,
                                                    name=f"pskn{bb}_{c}_{h}"))
                    for bb in range(2):
                        for h in heads:
                            nc.tensor.transpose(psqn[h][bb][:],
                                                QT[h][:, 128 * bb:128 * (bb + 1)],
                                                idb[h])
                        for h in heads:
                            nc.tensor.transpose(pskn[h][bb][:],
                                                KT[h][:, 128 * bb:128 * (bb + 1)],
                                                idb[h])
                    for bb in range(2):
                        for h in heads:
                            qnb = work.tile([128, 64], BF16, tag=f"qn{bb}{h}", bufs=2,
                                            name=f"qn{bb}_{c}_{h}")
                            cp(qnb[:], psqn[h][bb][:])
                            qn[h].append(qnb)
                            knb = work.tile([128, 64], BF16, tag=f"kn{bb}{h}", bufs=2,
                                            name=f"kn{bb}_{c}_{h}")
                            cp(knb[:], pskn[h][bb][:])
                            kn[h].append(knb)

                    # masks / casts of N and M
                    n0sb, n1sbR, triuN0L, triuN1, smM0L, m0R, smM1 = (
                        {}, {}, {}, {}, {}, {}, {})
                    for h in heads:
                        n0sb[h] = work.tile([128, 256], BF16, tag=f"n0sb{h}", bufs=2,
                                            name=f"n0sb_{c}_{h}")
                        cp(n0sb[h][:], psNM[h][:, 0:256])
                        n1sbR[h] = work.tile([128, 128], BF16, tag=f"n1sbR{h}", bufs=2,
                                             name=f"n1sbR_{c}_{h}")
                        cp(n1sbR[h][:], psNM[h][:, 256:384])
                        triuN0L[h] = work.tile([128, 128], BF16, tag=f"tN0{h}", bufs=2,
                                               name=f"tN0_{c}_{h}")
                        nc.vector.tensor_mul(triuN0L[h][:], psNM[h][:, 0:128], mtr[:])
                        triuN1[h] = work.tile([128, 128], BF16, tag=f"tN1{h}", bufs=2,
                                              name=f"tN1_{c}_{h}")
                        nc.vector.tensor_mul(triuN1[h][:], psNM[h][:, 256:384], mtr[:])
                        smM0L[h] = work.tile([128, 128], BF16, tag=f"sM0{h}", bufs=2,
                                             name=f"sM0_{c}_{h}")
                        nc.vector.tensor_mul(smM0L[h][:], psM[h][:, 0:128], mst[:])
                        m0R[h] = work.tile([128, 128], BF16, tag=f"m0R{h}", bufs=2,
                                           name=f"m0R_{c}_{h}")
                        cp(m0R[h][:], psM[h][:, 128:256])
                        smM1[h] = work.tile([128, 128], BF16, tag=f"sM1{h}", bufs=2,
                                            name=f"sM1_{c}_{h}")
                        nc.vector.tensor_mul(smM1[h][:], psM[h][:, 256:384], mst[:])

                    # QST = S0 @ QT per head, pair-packed [128, 256]
                    qstsb = None
                    if c > 0:
                        psQST = pps.tile([128, L], F32, tag="ps", bufs=3,
                                         name=f"psQST_{c}_{p}")
                        for h in heads:
                            po = 64 * (h & 1)
                            nc.tensor.matmul(psQST[po:po + 64, :],
                                             S[p][po:po + 64, :], QT[h],
                                             start=True, stop=True)
                        qstsb = work.tile([128, L], BF16, tag=f"qst{p}", bufs=2,
                                          name=f"qst_{c}_{p}")
                        cp(qstsb[:], psQST[:])

                    # AT = [AT0 (r 0:128, t 0:256) | AT1 right (r 128:256, t 128:256)]
                    psAT, at0, at1R = {}, {}, {}
                    for h in heads:
                        psAT[h] = pps.tile([128, 384], F32, tag="pb", bufs=5,
                                           name=f"psAT_{c}_{h}")
                    for h in heads:
                        nc.tensor.matmul(psAT[h][:, 0:256], triuN0L[h][:], n0sb[h][:],
                                         start=True, stop=(c == 0))
                    if c > 0:
                        for h in heads:
                            po = 64 * (h & 1)
                            nc.tensor.matmul(psAT[h][:, 0:256],
                                             qstsb[po:po + 64, 0:128], QT[h],
                                             start=False, stop=True)
                    for h in heads:
                        nc.tensor.matmul(psAT[h][:, 256:384], n0sb[h][:, 128:256],
                                         n0sb[h][:, 128:256], start=True, stop=False)
                    for h in heads:
                        nc.tensor.matmul(psAT[h][:, 256:384], triuN1[h][:],
                                         n1sbR[h][:], start=False, stop=(c == 0))
                    if c > 0:
                        for h in heads:
                            po = 64 * (h & 1)
                            nc.tensor.matmul(psAT[h][:, 256:384],
                                             qstsb[po:po + 64, 128:256],
                                             QT[h][:, 128:256],
                                             start=False, stop=True)
                    for h in heads:
                        at0[h] = work.tile([128, 256], BF16, tag=f"at0{h}", bufs=2,
                                           name=f"at0_{c}_{h}")
                        nc.vector.tensor_mul(at0[h][:], psAT[h][:, 0:256], mt0[:])
                        at1R[h] = work.tile([128, 128], BF16, tag=f"at1{h}", bufs=2,
                                            name=f"at1_{c}_{h}")
                        nc.vector.tensor_mul(at1R[h][:], psAT[h][:, 256:384], mtr[:])

                    # oT = V^T tril(AT) + (D0 Q^T), pair-packed [128, 256]
                    psO = pps.tile([128, L], F32, tag="ps", bufs=3,
                                   name=f"psO_{c}_{p}")
                    for h in heads:
                        po = 64 * (h & 1)
                        nc.tensor.matmul(psO[po:po + 64, :], Vbs[h][0], at0[h][:],
                                         start=True, stop=False)
                    for h in heads:
                        po = 64 * (h & 1)
                        nc.tensor.matmul(psO[po:po + 64, 128:256], Vbs[h][1],
                                         at1R[h][:], start=False, stop=(c == 0))
                    if c > 0:
                        for h in heads:
                            po = 64 * (h & 1)
                            nc.tensor.matmul(psO[po:po + 64, :],
                                             Dst[p][po:po + 64, :], QT[h],
                                             start=False, stop=True)
                    cp(ot[p][:], psO[:])

                    # dS = K^T K, dC = Q^T V (pair-packed [128, 64])
                    psS = pps.tile([128, 64], F32, tag="ps", bufs=3,
                                   name=f"psS_{c}_{p}")
                    psC = pps.tile([128, 64], F32, tag="ps", bufs=3,
                                   name=f"psC_{c}_{p}")
                    for bb in range(2):
                        for h in heads:
                            po = 64 * (h & 1)
                            nc.tensor.matmul(psS[po:po + 64, :], kn[h][bb][:],
                                             kn[h][bb][:],
                                             start=(bb == 0), stop=(bb == 1))
                        for h in heads:
                            po = 64 * (h & 1)
                            nc.tensor.matmul(psC[po:po + 64, :], qn[h][bb][:],
                                             Vbs[h][bb],
                                             start=(bb == 0), stop=(bb == 1))

                    # W2 = stril(KQ^T) V  per head: psW = [W2(s 0:128) | W2(s 128:256)]
                    wsb, psWs = {}, {}
                    for h in heads:
                        psWs[h] = pps.tile([128, 128], F32, tag="ps", bufs=3,
                                           name=f"psW_{c}_{h}")
                    for h in heads:
                        nc.tensor.matmul(psWs[h][:, 0:64], smM0L[h][:], Vbs[h][0],
                                         start=True, stop=True)
                    for h in heads:
                        nc.tensor.matmul(psWs[h][:, 64:128], m0R[h][:], Vbs[h][0],
                                         start=True, stop=False)
                    for h in heads:
                        nc.tensor.matmul(psWs[h][:, 64:128], smM1[h][:], Vbs[h][1],
                                         start=False, stop=True)
                    for h in heads:
                        wsb[h] = work.tile([128, 128], BF16, tag=f"wsb{h}", bufs=2,
                                           name=f"wsb_{c}_{h}")
                        cp(wsb[h][:], psWs[h][:])

                    # Gamma = K^T W2 (+ dS C0), pair-packed
                    dssb = None
                    if c > 0:
                        dssb = work.tile([128, 64], BF16, tag=f"dssb{p}", bufs=2,
                                         name=f"dssb_{c}_{p}")
                        cp(dssb[:], psS[:])
                    psG = pps.tile([128, 64], F32, tag="ps", bufs=3,
                                   name=f"psG_{c}_{p}")
                    for h in heads:
                        po = 64 * (h & 1)
                        nc.tensor.matmul(psG[po:po + 64, :], kn[h][0][:],
                                         wsb[h][:, 0:64], start=True, stop=False)
                    for h in heads:
                        po = 64 * (h & 1)
                        nc.tensor.matmul(psG[po:po + 64, :], kn[h][1][:],
                                         wsb[h][:, 64:128], start=False, stop=(c == 0))
                    if c > 0:
                        for h in heads:
                            po = 64 * (h & 1)
                            nc.tensor.matmul(psG[po:po + 64, :], dssb[po:po + 64, :],
                                             C[p][po:po + 64, :],
                                             start=False, stop=True)

                    # new states (one DVE op per state for the pair)
                    Snew = spool.tile([128, 64], BF16, tag=f"S{p}", bufs=2,
                                      name=f"S{p}_{c}")
                    Cnew = spool.tile([128, 64], BF16, tag=f"C{p}", bufs=2,
                                      name=f"C{p}_{c}")
                    Gnew = spool.tile([128, 64], BF16, tag=f"G{p}", bufs=2,
                                      name=f"G{p}_{c}")
                    if c > 0:
                        nc.vector.tensor_add(Snew[:], S[p][:], psS[:])
                        nc.vector.tensor_add(Cnew[:], C[p][:], psC[:])
                        nc.vector.tensor_add(Gnew[:], G[p][:], psG[:])
                    else:
                        nc.vector.tensor_copy(Snew[:], psS[:])
                        nc.vector.tensor_copy(Cnew[:], psC[:])
                        nc.vector.tensor_copy(Gnew[:], psG[:])
                    psD = pps.tile([128, 64], F32, tag="ps", bufs=3,
                                   name=f"psD_{c}_{p}")
                    for h in heads:
                        po = 64 * (h & 1)
                        nc.tensor.matmul(psD[po:po + 64, :], Snew[po:po + 64, :],
                                         Cnew[po:po + 64, :], start=True, stop=True)
                    Dnew = spool.tile([128, 64], BF16, tag=f"D{p}", bufs=2,
                                      name=f"D{p}_{c}")
                    nc.vector.tensor_sub(Dnew[:], psD[:], Gnew[:])
                    S[p], C[p], G[p], Dst[p] = Snew, Cnew, Gnew, Dnew

                # ---------- phase 3: output projection for this chunk ----------
                for bb in range(2):
                    for ncol in range(2):
                        pso = pps.tile([128, 512], F32, tag="pb", bufs=5,
                                       name=f"pso{bb}{ncol}_{c}")
                        nc.tensor.matmul(pso[:], ot[0][:, 128 * bb:128 * (bb + 1)],
                                         wo_sb[0][:, 512 * ncol:512 * (ncol + 1)],
                                         start=True, stop=False)
                        nc.tensor.matmul(pso[:], ot[1][:, 128 * bb:128 * (bb + 1)],
                                         wo_sb[1][:, 512 * ncol:512 * (ncol + 1)],
                                         start=False, stop=True)
                        osb = work.tile([128, 512], BF16, tag="osb", bufs=3,
                                        name=f"osb{bb}{ncol}_{c}")
                        cp(osb[:], pso[:])
                        nc.scalar.dma_start(
                            out=out_d[t0 + 128 * bb:t0 + 128 * (bb + 1),
                                      512 * ncol:512 * (ncol + 1)],
                            in_=osb[:])

    nc.compile()
    return nc


def _masks():
    p = np.arange(128)[:, None]
    f = np.arange(128)[None, :]
    triu = (f >= p).astype(np.float32)
    striu = (f > p).astype(np.float32)
    ones = np.ones((128, 128), np.float32)
    return {
        "ident": np.eye(128, dtype=BF),
        "mtr": triu,
        "mst": striu,
        "mt0": np.concatenate([triu, ones], axis=1),
    }


_NC_CACHE = {}


def get_nc(T=2048):
    if T not in _NC_CACHE:
        _NC_CACHE[T] = build(T)
    return _NC_CACHE[T]


def kernel(x, W_q, W_k, W_v, W_o):
    T = x.shape[1]
    nc = get_nc(T)
    masks = _masks()
    xts = [np.ascontiguousarray(x[b].T).astype(BF) for b in range(2)]
    in_maps = []
    for cc in range(8):
        b, g = cc // 4, cc % 4
        im = {
            "xt": xts[b],
            "wq": np.ascontiguousarray(W_q[:, DL * g:DL * (g + 1)]).astype(BF),
            "wk": np.ascontiguousarray(W_k[:, DL * g:DL * (g + 1)]).astype(BF),
            "wv": np.ascontiguousarray(W_v[:, DL * g:DL * (g + 1)]).astype(BF),
            "wo": np.ascontiguousarray(W_o[DL * g:DL * (g + 1), :]).astype(BF),
        }
        im.update(masks)
        in_maps.append(im)
    res = run_bass_kernel_spmd(nc, in_maps, list(range(8)))
    global _last_res
    _last_res = res
    out = np.zeros((2, T, D), np.float32)
    for cc in range(8):
        out[cc // 4] += np.asarray(res.results[cc]["out"], dtype=np.float32)
    return out


# revision 7
# speedup vs baseline: 1.1648x; 1.1648x over previous
"""Trainium2 Bass kernel for nn_HLALayer (higher-order linear attention).

Math: the reference scan
    k_C = k_t @ C;  G += k_t k_t^T C;  S += k_t k_t^T;  C += q_t v_t^T
    o_t = q_t @ (S C - G)
admits a chunked closed form (chunk L), carrying only S and Z = S C - G:
    o_chunk = Q @ Z0 + tril(Q S0 Q^T + A tril(A)^T) @ V,   A = Q K^T
    S1 = S0 + K^T K
    Z1 = Z0 + S0 (Q^T V) + K^T (tril(K Q^T, incl diag) V)

v3: bf16 operands (PSUM accumulation stays f32) -> FWL weight loads,
halved DMA; host passes x pre-transposed (no PE transposes of x); trimmed
triangular matmuls; head-pair packing of 64-wide matmuls onto disjoint PE
row/col groups; single shared transpose PSUM + one cast per head.

Sharding: 8 cores = (batch b in {0,1}) x (head-group g in {0..3}, 4 heads
each).  Each core projects x[b] with its weight column/row slices, runs the
chunked scan for its 4 heads, applies its W_o row-slice -> partial [T, D]
bf16 output; the host sums the 4 partials per batch in f32.
"""

import numpy as np
import sys

sys.path.insert(0, "/opt/trn_rl_repo")

import ml_dtypes
import concourse.bacc as bacc
import concourse.mybir as mybir
from concourse.bass_utils import run_bass_kernel_spmd
from concourse.tile import TileContext

F32 = mybir.dt.float32
BF16 = mybir.dt.bfloat16
BF = ml_dtypes.bfloat16

D = 1024          # model dim
DL = 256          # per-core projection width (4 heads x 64)
DK = 64           # head dim
L = 256           # chunk length
NHL = 4           # local heads per core


def build(T=2048):
    NCH = T // L
    nc = bacc.Bacc("TRN2", target_bir_lowering=False)

    xt_in = nc.declare_dram_parameter("xt", [D, T], BF16, isOutput=False)
    wq_in = nc.declare_dram_parameter("wq", [D, DL], BF16, isOutput=False)
    wk_in = nc.declare_dram_parameter("wk", [D, DL], BF16, isOutput=False)
    wv_in = nc.declare_dram_parameter("wv", [D, DL], BF16, isOutput=False)
    wo_in = nc.declare_dram_parameter("wo", [DL, D], BF16, isOutput=False)
    id_in = nc.declare_dram_parameter("ident", [128, 128], BF16, isOutput=False)
    mtr_in = nc.declare_dram_parameter("mtr", [128, 128], F32, isOutput=False)  # triu
    mtl_in = nc.declare_dram_parameter("mtl", [128, 128], F32, isOutput=False)  # tril
    mt0_in = nc.declare_dram_parameter("mt0", [128, 256], F32, isOutput=False)  # [triu|1]
    out_d = nc.declare_dram_parameter("out", [T, D], BF16, isOutput=True)

    ncp = 0  # copy-engine round robin counter

    with TileContext(nc) as tc:
        with tc.tile_pool(name="const", bufs=1) as cpool, \
             tc.tile_pool(name="work", bufs=2) as work, \
             tc.tile_pool(name="spool", bufs=2) as spool, \
             tc.tile_pool(name="pp", bufs=2, space="PSUM") as pps:

            def cp(out_ap, in_ap):
                """plain copy/cast, alternating DVE / ACT to balance load"""
                nonlocal ncp
                ncp += 1
                if ncp % 2 == 0:
                    nc.vector.tensor_copy(out_ap, in_ap)
                else:
                    nc.scalar.copy(out_ap, in_ap)

            # ---- constants / weights (gpsimd=SWDGE queue, keeps HWDGE free) ----
            ident = cpool.tile([128, 128], BF16)
            nc.gpsimd.dma_start(out=ident[:], in_=id_in[:])
            mtr = cpool.tile([128, 128], F32)
            nc.gpsimd.dma_start(out=mtr[:], in_=mtr_in[:])
            mtl = cpool.tile([128, 128], F32)
            nc.gpsimd.dma_start(out=mtl[:], in_=mtl_in[:])
            mt0 = cpool.tile([128, 256], F32)
            nc.gpsimd.dma_start(out=mt0[:], in_=mt0_in[:])

            wq_sb, wk_sb, wv_sb = [], [], []
            for j in range(8):
                wqt = cpool.tile([128, DL], BF16, name=f"wq{j}")
                nc.gpsimd.dma_start(out=wqt[:], in_=wq_in[128 * j:128 * (j + 1), :])
                wq_sb.append(wqt)
                wkt = cpool.tile([128, DL], BF16, name=f"wk{j}")
                nc.gpsimd.dma_start(out=wkt[:], in_=wk_in[128 * j:128 * (j + 1), :])
                wk_sb.append(wkt)
                wvt = cpool.tile([128, DL], BF16, name=f"wv{j}")
                nc.gpsimd.dma_start(out=wvt[:], in_=wv_in[128 * j:128 * (j + 1), :])
                wv_sb.append(wvt)
            wo_sb = []
            for m in range(2):
                wot = cpool.tile([128, D], BF16, name=f"wo{m}")
                nc.gpsimd.dma_start(out=wot[:], in_=wo_in[128 * m:128 * (m + 1), :])
                wo_sb.append(wot)

            # per-pair states (h0 at partitions 0:64, h1 at 64:128), bf16
            S = [None] * 2
            Z = [None] * 2

            for c in range(NCH):
                t0 = L * c
                # ---------- phase 1: load x^T tiles, project ----------
                xt = []
                for j in range(8):
                    xtj = work.tile([128, L], BF16, tag=f"xt{j}", bufs=2,
                                    name=f"xt{j}_{c}")
                    nc.sync.dma_start(out=xtj[:],
                                      in_=xt_in[128 * j:128 * (j + 1), t0:t0 + L])
                    xt.append(xtj)

                qt, kt = [], []
                for m in range(2):
                    psq = pps.tile([128, L], F32, tag="pb", bufs=5, name=f"psq{m}_{c}")
                    for j in range(8):
                        nc.tensor.matmul(psq[:], wq_sb[j][:, 128 * m:128 * (m + 1)],
                                         xt[j][:], start=(j == 0), stop=(j == 7))
                    qtm = work.tile([128, L], BF16, tag=f"qt{m}", bufs=2,
                                    name=f"qt{m}_{c}")
                    cp(qtm[:], psq[:])
                    qt.append(qtm)
                    psk = pps.tile([128, L], F32, tag="pb", bufs=5, name=f"psk{m}_{c}")
                    for j in range(8):
                        nc.tensor.matmul(psk[:], wk_sb[j][:, 128 * m:128 * (m + 1)],
                                         xt[j][:], start=(j == 0), stop=(j == 7))
                    ktm = work.tile([128, L], BF16, tag=f"kt{m}", bufs=2,
                                    name=f"kt{m}_{c}")
                    cp(ktm[:], psk[:])
                    kt.append(ktm)
                vt = []
                for bb in range(2):
                    psv = pps.tile([128, DL], F32, tag="pb", bufs=5, name=f"psv{bb}_{c}")
                    for j in range(8):
                        nc.tensor.matmul(psv[:], xt[j][:, 128 * bb:128 * (bb + 1)],
                                         wv_sb[j][:], start=(j == 0), stop=(j == 7))
                    vtb = work.tile([128, DL], BF16, tag=f"vt{bb}", bufs=2,
                                    name=f"vt{bb}_{c}")
                    cp(vtb[:], psv[:])
                    vt.append(vtb)

                # per-chunk output tiles (oT layout [dv, t]; p=0: heads 0,1)
                ot = [work.tile([128, L], BF16, tag=f"ot{m}", bufs=2,
                                name=f"ot{m}_{c}") for m in range(2)]

                # ---------- phase 2: chunked scan, head pairs ----------
                for p in range(2):
                    heads = (2 * p, 2 * p + 1)
                    QT, KT, Vbs, idb = {}, {}, {}, {}
                    for h in heads:
                        m, po = h >> 1, 64 * (h & 1)
                        QT[h] = qt[m][po:po + 64, :]
                        KT[h] = kt[m][po:po + 64, :]
                        Vbs[h] = [vt[bb][:, 64 * h:64 * h + 64] for bb in range(2)]
                        idb[h] = ident[po:po + 64, po:po + 64]

                    # N products: psNM = [N0 full (256) | N1 right (128)]
                    # M products: psM = [M0 left (128) | M1 full (256)]
                    psNM, psM = {}, {}
                    for h in heads:
                        psNM[h] = pps.tile([128, 384], F32, tag="pb", bufs=5,
                                           name=f"psNM_{c}_{h}")
                        nc.tensor.matmul(psNM[h][:, 0:256], KT[h][:, 0:128], QT[h],
                                         start=True, stop=True)
                        nc.tensor.matmul(psNM[h][:, 256:384], KT[h][:, 128:256],
                                         QT[h][:, 128:256], start=True, stop=True)
                    for h in heads:
                        psM[h] = pps.tile([128, 384], F32, tag="pb", bufs=5,
                                          name=f"psM_{c}_{h}")
                        nc.tensor.matmul(psM[h][:, 0:128], QT[h][:, 0:128],
                                         KT[h][:, 0:128], start=True, stop=True)
                        nc.tensor.matmul(psM[h][:, 128:384], QT[h][:, 128:256],
                                         KT[h][:], start=True, stop=True)

                    # natural q/k via PE transpose into one shared PSUM tile per
                    # head: qkn = [qn0 | qn1 | kn0 | kn1], one cast per head
                    qkn = {}
                    for h in heads:
                        po = 64 * (h & 1)
                        psqk = pps.tile([128, 256], BF16, tag="ps", bufs=3,
                                        name=f"psqk_{c}_{h}")
                        for bb in range(2):
                            nc.tensor.transpose(psqk[:, 64 * bb:64 * (bb + 1)],
                                                QT[h][:, 128 * bb:128 * (bb + 1)],
                                                idb[h])
                            nc.tensor.transpose(psqk[:, 128 + 64 * bb:192 + 64 * bb],
                                                KT[h][:, 128 * bb:128 * (bb + 1)],
                                                idb[h])
                        qknh = work.tile([128, 256], BF16, tag=f"qkn{h}", bufs=2,
                                         name=f"qkn_{c}_{h}")
                        cp(qknh[:], psqk[:])
                        qkn[h] = qknh
                    qn = {h: [qkn[h][:, 0:64], qkn[h][:, 64:128]] for h in heads}
                    kn = {h: [qkn[h][:, 128:192], qkn[h][:, 192:256]] for h in heads}

                    # masks / casts of N and M
                    n0sb, n1sbR, triuN0L, triuN1, trilM0L, m1L, trilM1R = (
                        {}, {}, {}, {}, {}, {}, {})
                    for h in heads:
                        n0sb[h] = work.tile([128, 256], BF16, tag=f"n0sb{h}", bufs=2,
                                            name=f"n0sb_{c}_{h}")
                        cp(n0sb[h][:], psNM[h][:, 0:256])
                        n1sbR[h] = work.tile([128, 128], BF16, tag=f"n1sbR{h}", bufs=2,
                                             name=f"n1sbR_{c}_{h}")
                        cp(n1sbR[h][:], psNM[h][:, 256:384])
                        triuN0L[h] = work.tile([128, 128], BF16, tag=f"tN0{h}", bufs=2,
                                               name=f"tN0_{c}_{h}")
                        nc.vector.tensor_mul(triuN0L[h][:], psNM[h][:, 0:128], mtr[:])
                        triuN1[h] = work.tile([128, 128], BF16, tag=f"tN1{h}", bufs=2,
                                              name=f"tN1_{c}_{h}")
                        nc.vector.tensor_mul(triuN1[h][:], psNM[h][:, 256:384], mtr[:])
                        trilM0L[h] = work.tile([128, 128], BF16, tag=f"lM0{h}", bufs=2,
                                               name=f"lM0_{c}_{h}")
                        nc.vector.tensor_mul(trilM0L[h][:], psM[h][:, 0:128], mtl[:])
                        m1L[h] = work.tile([128, 128], BF16, tag=f"m1L{h}", bufs=2,
                                           name=f"m1L_{c}_{h}")
                        cp(m1L[h][:], psM[h][:, 128:256])
                        trilM1R[h] = work.tile([128, 128], BF16, tag=f"lM1{h}", bufs=2,
                                               name=f"lM1_{c}_{h}")
                        nc.vector.tensor_mul(trilM1R[h][:], psM[h][:, 256:384], mtl[:])

                    # QST = S0 @ QT per head, pair-packed [128, 256]
                    qstsb = None
                    if c > 0:
                        psQST = pps.tile([128, L], F32, tag="ps", bufs=3,
                                         name=f"psQST_{c}_{p}")
                        for h in heads:
                            po = 64 * (h & 1)
                            nc.tensor.matmul(psQST[po:po + 64, :],
                                             S[p][po:po + 64, :], QT[h],
                                             start=True, stop=True)
                        qstsb = work.tile([128, L], BF16, tag=f"qst{p}", bufs=2,
                                          name=f"qst_{c}_{p}")
                        cp(qstsb[:], psQST[:])

                    # AT = [AT0 (r 0:128, t 0:256) | AT1 right (r 128:256, t 128:256)]
                    psAT, at0, at1R = {}, {}, {}
                    for h in heads:
                        po = 64 * (h & 1)
                        psAT[h] = pps.tile([128, 384], F32, tag="pb", bufs=5,
                                           name=f"psAT_{c}_{h}")
                        nc.tensor.matmul(psAT[h][:, 0:256], triuN0L[h][:], n0sb[h][:],
                                         start=True, stop=(c == 0))
                        if c > 0:
                            nc.tensor.matmul(psAT[h][:, 0:256],
                                             qstsb[po:po + 64, 0:128], QT[h],
                                             start=False, stop=True)
                        nc.tensor.matmul(psAT[h][:, 256:384], n0sb[h][:, 128:256],
                                         n0sb[h][:, 128:256], start=True, stop=False)
                        nc.tensor.matmul(psAT[h][:, 256:384], triuN1[h][:],
                                         n1sbR[h][:], start=False, stop=(c == 0))
                        if c > 0:
                            nc.tensor.matmul(psAT[h][:, 256:384],
                                             qstsb[po:po + 64, 128:256],
                                             QT[h][:, 128:256],
                                             start=False, stop=True)
                    for h in heads:
                        at0[h] = work.tile([128, 256], BF16, tag=f"at0{h}", bufs=2,
                                           name=f"at0_{c}_{h}")
                        nc.vector.tensor_mul(at0[h][:], psAT[h][:, 0:256], mt0[:])
                        at1R[h] = work.tile([128, 128], BF16, tag=f"at1{h}", bufs=2,
                                            name=f"at1_{c}_{h}")
                        nc.vector.tensor_mul(at1R[h][:], psAT[h][:, 256:384], mtr[:])

                    # oT = V^T tril(AT) + (Z0^T Q^T), pair-packed [128, 256]
                    psO = pps.tile([128, L], F32, tag="ps", bufs=3,
                                   name=f"psO_{c}_{p}")
                    for h in heads:
                        po = 64 * (h & 1)
                        nc.tensor.matmul(psO[po:po + 64, :], Vbs[h][0], at0[h][:],
                                         start=True, stop=False)
                        nc.tensor.matmul(psO[po:po + 64, 128:256], Vbs[h][1],
                                         at1R[h][:], start=False, stop=(c == 0))
                        if c > 0:
                            nc.tensor.matmul(psO[po:po + 64, :],
                                             Z[p][po:po + 64, :], QT[h],
                                             start=False, stop=True)
                    cp(ot[p][:], psO[:])

                    # dS = K^T K, dC = Q^T V (pair-packed [128, 64+64])
                    psSC = pps.tile([128, 128], F32, tag="ps", bufs=3,
                                    name=f"psSC_{c}_{p}")
                    for h in heads:
                        po = 64 * (h & 1)
                        nc.tensor.matmul(psSC[po:po + 64, 0:64], kn[h][0], kn[h][0],
                                         start=True, stop=False)
                        nc.tensor.matmul(psSC[po:po + 64, 0:64], kn[h][1], kn[h][1],
                                         start=False, stop=True)
                        if c > 0:
                            nc.tensor.matmul(psSC[po:po + 64, 64:128], qn[h][0],
                                             Vbs[h][0], start=True, stop=False)
                            nc.tensor.matmul(psSC[po:po + 64, 64:128], qn[h][1],
                                             Vbs[h][1], start=False, stop=True)

                    # W2 = tril(KQ^T, incl diag) V per head:
                    # psW = [W2(s 0:128) | W2(s 128:256)]
                    wsb = {}
                    for h in heads:
                        psW = pps.tile([128, 128], F32, tag="ps", bufs=3,
                                       name=f"psW_{c}_{h}")
                        nc.tensor.matmul(psW[:, 0:64], trilM0L[h][:], Vbs[h][0],
                                         start=True, stop=False)
                        nc.tensor.matmul(psW[:, 0:64], m1L[h][:], Vbs[h][1],
                                         start=False, stop=True)
                        nc.tensor.matmul(psW[:, 64:128], trilM1R[h][:], Vbs[h][1],
                                         start=True, stop=True)
                        wsb[h] = work.tile([128, 128], BF16, tag=f"wsb{h}", bufs=2,
                                           name=f"wsb_{c}_{h}")
                        cp(wsb[h][:], psW[:])

                    # dZ = K^T W2 (+ S0 dC), pair-packed
                    dcsb = None
                    if c > 0:
                        dcsb = work.tile([128, 64], BF16, tag=f"dcsb{p}", bufs=2,
                                         name=f"dcsb_{c}_{p}")
                        cp(dcsb[:], psSC[:, 64:128])
                    psZ = pps.tile([128, 64], F32, tag="ps", bufs=3,
                                   name=f"psZ_{c}_{p}")
                    for h in heads:
                        po = 64 * (h & 1)
                        nc.tensor.matmul(psZ[po:po + 64, :], kn[h][0],
                                         wsb[h][:, 0:64], start=True, stop=False)
                        nc.tensor.matmul(psZ[po:po + 64, :], kn[h][1],
                                         wsb[h][:, 64:128], start=False, stop=(c == 0))
                        if c > 0:
                            nc.tensor.matmul(psZ[po:po + 64, :], S[p][po:po + 64, :],
                                             dcsb[po:po + 64, :],
                                             start=False, stop=True)

                    # new states (one DVE op per state for the pair)
                    Snew = spool.tile([128, 64], BF16, tag=f"S{p}", bufs=2,
                                      name=f"S{p}_{c}")
                    Znew = spool.tile([128, 64], BF16, tag=f"Z{p}", bufs=2,
                                      name=f"Z{p}_{c}")
                    if c > 0:
                        nc.vector.tensor_add(Snew[:], S[p][:], psSC[:, 0:64])
                        nc.vector.tensor_add(Znew[:], Z[p][:], psZ[:])
                    else:
                        nc.vector.tensor_copy(Snew[:], psSC[:, 0:64])
                        nc.vector.tensor_copy(Znew[:], psZ[:])
                    S[p], Z[p] = Snew, Znew

                # ---------- phase 3: output projection for this chunk ----------
                for bb in range(2):
                    for ncol in range(2):
                        pso = pps.tile([128, 512], F32, tag="pb", bufs=5,
                                       name=f"pso{bb}{ncol}_{c}")
                        nc.tensor.matmul(pso[:], ot[0][:, 128 * bb:128 * (bb + 1)],
                                         wo_sb[0][:, 512 * ncol:512 * (ncol + 1)],
                                         start=True, stop=False)
                        nc.tensor.matmul(pso[:], ot[1][:, 128 * bb:128 * (bb + 1)],
                                         wo_sb[1][:, 512 * ncol:512 * (ncol + 1)],
                                         start=False, stop=True)
                        osb = work.tile([128, 512], BF16, tag="osb", bufs=3,
                                        name=f"osb{bb}{ncol}_{c}")
                        cp(osb[:], pso[:])
                        nc.scalar.dma_start(
                            out=out_d[t0 + 128 * bb:t0 + 128 * (bb + 1),
                                      512 * ncol:512 * (ncol + 1)],
                            in_=osb[:])

    nc.compile()
    return nc


def _masks():
    p = np.arange(128)[:, None]
    f = np.arange(128)[None, :]
    triu = (f >= p).astype(np.float32)
    tril = (f <= p).astype(np.float32)
    ones = np.ones((128, 128), np.float32)
    return {
        "ident": np.eye(128, dtype=BF),
        "mtr": triu,
        "mtl": tril,
        "mt0": np.concatenate([triu, ones], axis=1),
    }


_NC_CACHE = {}


def get_nc(T=2048):
    if T not in _NC_CACHE:
        _NC_CACHE[T] = build(T)
    return _NC_CACHE[T]


def kernel(x, W_q, W_k, W_v, W_o):
    T = x.shape[1]
    nc = get_nc(T)
    masks = _masks()
    xts = [np.ascontiguousarray(x[b].T).astype(BF) for b in range(2)]
    in_maps = []
    for cc in range(8):
        b, g = cc // 4, cc % 4
        im = {
            "xt": xts[b],
            "wq": np.ascontiguousarray(W_q[:, DL * g:DL * (g + 1)]).astype(BF),
            "wk": np.ascontiguousarray(W_k[:, DL * g:DL * (g + 1)]).astype(BF),
            "wv": np.ascontiguousarray(W_v[:, DL * g:DL * (g + 1)]).astype(BF),
            "wo": np.ascontiguousarray(W_o[DL * g:DL * (g + 1), :]).astype(BF),
        }
        im.update(masks)
        in_maps.append(im)
    res = run_bass_kernel_spmd(nc, in_maps, list(range(8)))
    global _last_res
    _last_res = res
    out = np.zeros((2, T, D), np.float32)
    for cc in range(8):
        out[cc // 4] += np.asarray(res.results[cc]["out"], dtype=np.float32)
    return out


# revision 8
# speedup vs baseline: 1.1680x; 1.0027x over previous
"""Trainium2 Bass kernel for nn_HLALayer (higher-order linear attention).

Math: the reference scan
    k_C = k_t @ C;  G += k_t k_t^T C;  S += k_t k_t^T;  C += q_t v_t^T
    o_t = q_t @ (S C - G)
admits a chunked closed form (chunk L), carrying only S and Z = S C - G:
    o_chunk = Q @ Z0 + tril(Q S0 Q^T + A tril(A)^T) @ V,   A = Q K^T
    S1 = S0 + K^T K
    Z1 = Z0 + S0 (Q^T V) + K^T (tril(K Q^T, incl diag) V)

v3: bf16 operands (PSUM accumulation stays f32) -> FWL weight loads,
halved DMA; host passes x pre-transposed (no PE transposes of x); trimmed
triangular matmuls; head-pair packing of 64-wide matmuls onto disjoint PE
row/col groups; single shared transpose PSUM + one cast per head.

Sharding: 8 cores = (batch b in {0,1}) x (head-group g in {0..3}, 4 heads
each).  Each core projects x[b] with its weight column/row slices, runs the
chunked scan for its 4 heads, applies its W_o row-slice -> partial [T, D]
bf16 output; the host sums the 4 partials per batch in f32.
"""

import numpy as np
import sys

sys.path.insert(0, "/opt/trn_rl_repo")

import ml_dtypes
import concourse.bacc as bacc
import concourse.mybir as mybir
from concourse.bass_utils import run_bass_kernel_spmd
from concourse.tile import TileContext

F32 = mybir.dt.float32
BF16 = mybir.dt.bfloat16
BF = ml_dtypes.bfloat16

D = 1024          # model dim
DL = 256          # per-core projection width (4 heads x 64)
DK = 64           # head dim
L = 256           # chunk length
NHL = 4           # local heads per core


def build(T=2048):
    NCH = T // L
    nc = bacc.Bacc("TRN2", target_bir_lowering=False)

    xt_in = nc.declare_dram_parameter("xt", [D, T], BF16, isOutput=False)
    wq_in = nc.declare_dram_parameter("wq", [D, DL], BF16, isOutput=False)
    wk_in = nc.declare_dram_parameter("wk", [D, DL], BF16, isOutput=False)
    wv_in = nc.declare_dram_parameter("wv", [D, DL], BF16, isOutput=False)
    wo_in = nc.declare_dram_parameter("wo", [DL, D], BF16, isOutput=False)
    id_in = nc.declare_dram_parameter("ident", [128, 128], BF16, isOutput=False)
    mtr_in = nc.declare_dram_parameter("mtr", [128, 128], F32, isOutput=False)  # triu
    mtl_in = nc.declare_dram_parameter("mtl", [128, 128], F32, isOutput=False)  # tril
    mt0_in = nc.declare_dram_parameter("mt0", [128, 256], F32, isOutput=False)  # [triu|1]
    out_d = nc.declare_dram_parameter("out", [T, D], BF16, isOutput=True)

    ncp = 0  # copy-engine round robin counter

    with TileContext(nc) as tc:
        with tc.tile_pool(name="const", bufs=1) as cpool, \
             tc.tile_pool(name="work", bufs=2) as work, \
             tc.tile_pool(name="spool", bufs=2) as spool, \
             tc.tile_pool(name="pp", bufs=2, space="PSUM") as pps:

            def cp(out_ap, in_ap):
                """plain copy/cast, alternating DVE / ACT to balance load"""
                nonlocal ncp
                ncp += 1
                if ncp % 2 == 0:
                    nc.vector.tensor_copy(out_ap, in_ap)
                else:
                    nc.scalar.copy(out_ap, in_ap)

            # ---- constants / weights (gpsimd=SWDGE queue, keeps HWDGE free) ----
            ident = cpool.tile([128, 128], BF16)
            nc.gpsimd.dma_start(out=ident[:], in_=id_in[:])
            mtr = cpool.tile([128, 128], F32)
            nc.gpsimd.dma_start(out=mtr[:], in_=mtr_in[:])
            mtl = cpool.tile([128, 128], F32)
            nc.gpsimd.dma_start(out=mtl[:], in_=mtl_in[:])
            mt0 = cpool.tile([128, 256], F32)
            nc.gpsimd.dma_start(out=mt0[:], in_=mt0_in[:])

            wq_sb, wk_sb, wv_sb = [], [], []
            for j in range(8):
                wqt = cpool.tile([128, DL], BF16, name=f"wq{j}")
                nc.gpsimd.dma_start(out=wqt[:], in_=wq_in[128 * j:128 * (j + 1), :])
                wq_sb.append(wqt)
                wkt = cpool.tile([128, DL], BF16, name=f"wk{j}")
                nc.gpsimd.dma_start(out=wkt[:], in_=wk_in[128 * j:128 * (j + 1), :])
                wk_sb.append(wkt)
                wvt = cpool.tile([128, DL], BF16, name=f"wv{j}")
                nc.gpsimd.dma_start(out=wvt[:], in_=wv_in[128 * j:128 * (j + 1), :])
                wv_sb.append(wvt)
            wo_sb = []
            for m in range(2):
                wot = cpool.tile([128, D], BF16, name=f"wo{m}")
                nc.gpsimd.dma_start(out=wot[:], in_=wo_in[128 * m:128 * (m + 1), :])
                wo_sb.append(wot)

            # per-pair states (h0 at partitions 0:64, h1 at 64:128), bf16
            S = [None] * 2
            Z = [None] * 2

            for c in range(NCH):
                t0 = L * c
                # ---------- phase 1: load x^T tiles, project ----------
                xt = []
                for j in range(8):
                    xtj = work.tile([128, L], BF16, tag=f"xt{j}", bufs=3,
                                    name=f"xt{j}_{c}")
                    nc.sync.dma_start(out=xtj[:],
                                      in_=xt_in[128 * j:128 * (j + 1), t0:t0 + L])
                    xt.append(xtj)

                qt, kt = [], []
                for m in range(2):
                    psq = pps.tile([128, L], F32, tag="pb", bufs=5, name=f"psq{m}_{c}")
                    for j in range(8):
                        nc.tensor.matmul(psq[:], wq_sb[j][:, 128 * m:128 * (m + 1)],
                                         xt[j][:], start=(j == 0), stop=(j == 7))
                    qtm = work.tile([128, L], BF16, tag=f"qt{m}", bufs=3,
                                    name=f"qt{m}_{c}")
                    cp(qtm[:], psq[:])
                    qt.append(qtm)
                    psk = pps.tile([128, L], F32, tag="pb", bufs=5, name=f"psk{m}_{c}")
                    for j in range(8):
                        nc.tensor.matmul(psk[:], wk_sb[j][:, 128 * m:128 * (m + 1)],
                                         xt[j][:], start=(j == 0), stop=(j == 7))
                    ktm = work.tile([128, L], BF16, tag=f"kt{m}", bufs=3,
                                    name=f"kt{m}_{c}")
                    cp(ktm[:], psk[:])
                    kt.append(ktm)
                vt = []
                for bb in range(2):
                    psv = pps.tile([128, DL], F32, tag="pb", bufs=5, name=f"psv{bb}_{c}")
                    for j in range(8):
                        nc.tensor.matmul(psv[:], xt[j][:, 128 * bb:128 * (bb + 1)],
                                         wv_sb[j][:], start=(j == 0), stop=(j == 7))
                    vtb = work.tile([128, DL], BF16, tag=f"vt{bb}", bufs=3,
                                    name=f"vt{bb}_{c}")
                    cp(vtb[:], psv[:])
                    vt.append(vtb)

                # per-chunk output tiles (oT layout [dv, t]; p=0: heads 0,1)
                ot = [work.tile([128, L], BF16, tag=f"ot{m}", bufs=3,
                                name=f"ot{m}_{c}") for m in range(2)]

                # ---------- phase 2: chunked scan, head pairs ----------
                for p in range(2):
                    heads = (2 * p, 2 * p + 1)
                    QT, KT, Vbs, idb = {}, {}, {}, {}
                    for h in heads:
                        m, po = h >> 1, 64 * (h & 1)
                        QT[h] = qt[m][po:po + 64, :]
                        KT[h] = kt[m][po:po + 64, :]
                        Vbs[h] = [vt[bb][:, 64 * h:64 * h + 64] for bb in range(2)]
                        idb[h] = ident[po:po + 64, po:po + 64]

                    # N products: psNM = [N0 full (256) | N1 right (128)]
                    # M products: psM = [M0 left (128) | M1 full (256)]
                    psNM, psM = {}, {}
                    for h in heads:
                        psNM[h] = pps.tile([128, 384], F32, tag="pb", bufs=5,
                                           name=f"psNM_{c}_{h}")
                        nc.tensor.matmul(psNM[h][:, 0:256], KT[h][:, 0:128], QT[h],
                                         start=True, stop=True)
                        nc.tensor.matmul(psNM[h][:, 256:384], KT[h][:, 128:256],
                                         QT[h][:, 128:256], start=True, stop=True)
                    for h in heads:
                        psM[h] = pps.tile([128, 384], F32, tag="pb", bufs=5,
                                          name=f"psM_{c}_{h}")
                        nc.tensor.matmul(psM[h][:, 0:128], QT[h][:, 0:128],
                                         KT[h][:, 0:128], start=True, stop=True)
                        nc.tensor.matmul(psM[h][:, 128:384], QT[h][:, 128:256],
                                         KT[h][:], start=True, stop=True)

                    # natural q/k via PE transpose into one shared PSUM tile per
                    # head: qkn = [qn0 | qn1 | kn0 | kn1], one cast per head
                    qkn = {}
                    for h in heads:
                        po = 64 * (h & 1)
                        psqk = pps.tile([128, 256], BF16, tag="ps", bufs=3,
                                        name=f"psqk_{c}_{h}")
                        for bb in range(2):
                            nc.tensor.transpose(psqk[:, 64 * bb:64 * (bb + 1)],
                                                QT[h][:, 128 * bb:128 * (bb + 1)],
                                                idb[h])
                            nc.tensor.transpose(psqk[:, 128 + 64 * bb:192 + 64 * bb],
                                                KT[h][:, 128 * bb:128 * (bb + 1)],
                                                idb[h])
                        qknh = work.tile([128, 256], BF16, tag=f"qkn{h}", bufs=2,
                                         name=f"qkn_{c}_{h}")
                        cp(qknh[:], psqk[:])
                        qkn[h] = qknh
                    qn = {h: [qkn[h][:, 0:64], qkn[h][:, 64:128]] for h in heads}
                    kn = {h: [qkn[h][:, 128:192], qkn[h][:, 192:256]] for h in heads}

                    # masks / casts of N and M
                    n0sb, n1sbR, triuN0L, triuN1, trilM0L, m1L, trilM1R = (
                        {}, {}, {}, {}, {}, {}, {})
                    for h in heads:
                        n0sb[h] = work.tile([128, 256], BF16, tag=f"n0sb{h}", bufs=2,
                                            name=f"n0sb_{c}_{h}")
                        cp(n0sb[h][:], psNM[h][:, 0:256])
                        n1sbR[h] = work.tile([128, 128], BF16, tag=f"n1sbR{h}", bufs=2,
                                             name=f"n1sbR_{c}_{h}")
                        cp(n1sbR[h][:], psNM[h][:, 256:384])
                        triuN0L[h] = work.tile([128, 128], BF16, tag=f"tN0{h}", bufs=2,
                                               name=f"tN0_{c}_{h}")
                        nc.vector.tensor_mul(triuN0L[h][:], psNM[h][:, 0:128], mtr[:])
                        triuN1[h] = work.tile([128, 128], BF16, tag=f"tN1{h}", bufs=2,
                                              name=f"tN1_{c}_{h}")
                        nc.vector.tensor_mul(triuN1[h][:], psNM[h][:, 256:384], mtr[:])
                        trilM0L[h] = work.tile([128, 128], BF16, tag=f"lM0{h}", bufs=2,
                                               name=f"lM0_{c}_{h}")
                        nc.vector.tensor_mul(trilM0L[h][:], psM[h][:, 0:128], mtl[:])
                        m1L[h] = work.tile([128, 128], BF16, tag=f"m1L{h}", bufs=2,
                                           name=f"m1L_{c}_{h}")
                        cp(m1L[h][:], psM[h][:, 128:256])
                        trilM1R[h] = work.tile([128, 128], BF16, tag=f"lM1{h}", bufs=2,
                                               name=f"lM1_{c}_{h}")
                        nc.vector.tensor_mul(trilM1R[h][:], psM[h][:, 256:384], mtl[:])

                    # QST = S0 @ QT per head, pair-packed [128, 256]
                    qstsb = None
                    if c > 0:
                        psQST = pps.tile([128, L], F32, tag="ps", bufs=3,
                                         name=f"psQST_{c}_{p}")
                        for h in heads:
                            po = 64 * (h & 1)
                            nc.tensor.matmul(psQST[po:po + 64, :],
                                             S[p][po:po + 64, :], QT[h],
                                             start=True, stop=True)
                        qstsb = work.tile([128, L], BF16, tag=f"qst{p}", bufs=2,
                                          name=f"qst_{c}_{p}")
                        cp(qstsb[:], psQST[:])

                    # AT = [AT0 (r 0:128, t 0:256) | AT1 right (r 128:256, t 128:256)]
                    psAT, at0, at1R = {}, {}, {}
                    for h in heads:
                        po = 64 * (h & 1)
                        psAT[h] = pps.tile([128, 384], F32, tag="pb", bufs=5,
                                           name=f"psAT_{c}_{h}")
                        nc.tensor.matmul(psAT[h][:, 0:256], triuN0L[h][:], n0sb[h][:],
                                         start=True, stop=(c == 0))
                        if c > 0:
                            nc.tensor.matmul(psAT[h][:, 0:256],
                                             qstsb[po:po + 64, 0:128], QT[h],
                                             start=False, stop=True)
                        nc.tensor.matmul(psAT[h][:, 256:384], n0sb[h][:, 128:256],
                                         n0sb[h][:, 128:256], start=True, stop=False)
                        nc.tensor.matmul(psAT[h][:, 256:384], triuN1[h][:],
                                         n1sbR[h][:], start=False, stop=(c == 0))
                        if c > 0:
                            nc.tensor.matmul(psAT[h][:, 256:384],
                                             qstsb[po:po + 64, 128:256],
                                             QT[h][:, 128:256],
                                             start=False, stop=True)
                    for h in heads:
                        at0[h] = work.tile([128, 256], BF16, tag=f"at0{h}", bufs=2,
                                           name=f"at0_{c}_{h}")
                        nc.vector.tensor_mul(at0[h][:], psAT[h][:, 0:256], mt0[:])
                        at1R[h] = work.tile([128, 128], BF16, tag=f"at1{h}", bufs=2,
                                            name=f"at1_{c}_{h}")
                        nc.vector.tensor_mul(at1R[h][:], psAT[h][:, 256:384], mtr[:])

                    # oT = V^T tril(AT) + (Z0^T Q^T), pair-packed [128, 256]
                    psO = pps.tile([128, L], F32, tag="ps", bufs=3,
                                   name=f"psO_{c}_{p}")
                    for h in heads:
                        po = 64 * (h & 1)
                        nc.tensor.matmul(psO[po:po + 64, :], Vbs[h][0], at0[h][:],
                                         start=True, stop=False)
                        nc.tensor.matmul(psO[po:po + 64, 128:256], Vbs[h][1],
                                         at1R[h][:], start=False, stop=(c == 0))
                        if c > 0:
                            nc.tensor.matmul(psO[po:po + 64, :],
                                             Z[p][po:po + 64, :], QT[h],
                                             start=False, stop=True)
                    cp(ot[p][:], psO[:])

                    # dS = K^T K, dC = Q^T V (pair-packed [128, 64+64])
                    psSC = pps.tile([128, 128], F32, tag="ps", bufs=3,
                                    name=f"psSC_{c}_{p}")
                    for h in heads:
                        po = 64 * (h & 1)
                        nc.tensor.matmul(psSC[po:po + 64, 0:64], kn[h][0], kn[h][0],
                                         start=True, stop=False)
                        nc.tensor.matmul(psSC[po:po + 64, 0:64], kn[h][1], kn[h][1],
                                         start=False, stop=True)
                        if c > 0:
                            nc.tensor.matmul(psSC[po:po + 64, 64:128], qn[h][0],
                                             Vbs[h][0], start=True, stop=False)
                            nc.tensor.matmul(psSC[po:po + 64, 64:128], qn[h][1],
                                             Vbs[h][1], start=False, stop=True)

                    # W2 = tril(KQ^T, incl diag) V per head:
                    # psW = [W2(s 0:128) | W2(s 128:256)]
                    wsb = {}
                    for h in heads:
                        psW = pps.tile([128, 128], F32, tag="ps", bufs=3,
                                       name=f"psW_{c}_{h}")
                        nc.tensor.matmul(psW[:, 0:64], trilM0L[h][:], Vbs[h][0],
                                         start=True, stop=False)
                        nc.tensor.matmul(psW[:, 0:64], m1L[h][:], Vbs[h][1],
                                         start=False, stop=True)
                        nc.tensor.matmul(psW[:, 64:128], trilM1R[h][:], Vbs[h][1],
                                         start=True, stop=True)
                        wsb[h] = work.tile([128, 128], BF16, tag=f"wsb{h}", bufs=2,
                                           name=f"wsb_{c}_{h}")
                        cp(wsb[h][:], psW[:])

                    # dZ = K^T W2 (+ S0 dC), pair-packed
                    dcsb = None
                    if c > 0:
                        dcsb = work.tile([128, 64], BF16, tag=f"dcsb{p}", bufs=2,
                                         name=f"dcsb_{c}_{p}")
                        cp(dcsb[:], psSC[:, 64:128])
                    psZ = pps.tile([128, 64], F32, tag="ps", bufs=3,
                                   name=f"psZ_{c}_{p}")
                    for h in heads:
                        po = 64 * (h & 1)
                        nc.tensor.matmul(psZ[po:po + 64, :], kn[h][0],
                                         wsb[h][:, 0:64], start=True, stop=False)
                        nc.tensor.matmul(psZ[po:po + 64, :], kn[h][1],
                                         wsb[h][:, 64:128], start=False, stop=(c == 0))
                        if c > 0:
                            nc.tensor.matmul(psZ[po:po + 64, :], S[p][po:po + 64, :],
                                             dcsb[po:po + 64, :],
                                             start=False, stop=True)

                    # new states (one DVE op per state for the pair)
                    Snew = spool.tile([128, 64], BF16, tag=f"S{p}", bufs=2,
                                      name=f"S{p}_{c}")
                    Znew = spool.tile([128, 64], BF16, tag=f"Z{p}", bufs=2,
                                      name=f"Z{p}_{c}")
                    if c > 0:
                        nc.vector.tensor_add(Snew[:], S[p][:], psSC[:, 0:64])
                        nc.vector.tensor_add(Znew[:], Z[p][:], psZ[:])
                    else:
                        nc.vector.tensor_copy(Snew[:], psSC[:, 0:64])
                        nc.vector.tensor_copy(Znew[:], psZ[:])
                    S[p], Z[p] = Snew, Znew

                # ---------- phase 3: output projection for this chunk ----------
                for bb in range(2):
                    for ncol in range(2):
                        pso = pps.tile([128, 512], F32, tag="ps", bufs=3,
                                       name=f"pso{bb}{ncol}_{c}")
                        nc.tensor.matmul(pso[:], ot[0][:, 128 * bb:128 * (bb + 1)],
                                         wo_sb[0][:, 512 * ncol:512 * (ncol + 1)],
                                         start=True, stop=False)
                        nc.tensor.matmul(pso[:], ot[1][:, 128 * bb:128 * (bb + 1)],
                                         wo_sb[1][:, 512 * ncol:512 * (ncol + 1)],
                                         start=False, stop=True)
                        osb = work.tile([128, 512], BF16, tag="osb", bufs=3,
                                        name=f"osb{bb}{ncol}_{c}")
                        cp(osb[:], pso[:])
                        nc.scalar.dma_start(
                            out=out_d[t0 + 128 * bb:t0 + 128 * (bb + 1),
                                      512 * ncol:512 * (ncol + 1)],
                            in_=osb[:])

    nc.compile()
    return nc


def _masks():
    p = np.arange(128)[:, None]
    f = np.arange(128)[None, :]
    triu = (f >= p).astype(np.float32)
    tril = (f <= p).astype(np.float32)
    ones = np.ones((128, 128), np.float32)
    return {
        "ident": np.eye(128, dtype=BF),
        "mtr": triu,
        "mtl": tril,
        "mt0": np.concatenate([triu, ones], axis=1),
    }


_NC_CACHE = {}


def get_nc(T=2048):
    if T not in _NC_CACHE:
        _NC_CACHE[T] = build(T)
    return _NC_CACHE[T]


def kernel(x, W_q, W_k, W_v, W_o):
    T = x.shape[1]
    nc = get_nc(T)
    masks = _masks()
    xts = [np.ascontiguousarray(x[b].T).astype(BF) for b in range(2)]
    in_maps = []
    for cc in range(8):
        b, g = cc // 4, cc % 4
        im = {
            "xt": xts[b],
            "wq": np.ascontiguousarray(W_q[:, DL * g:DL * (g + 1)]).astype(BF),
            "wk": np.ascontiguousarray(W_k[:, DL * g:DL * (g + 1)]).astype(BF),
            "wv": np.ascontiguousarray(W_v[:, DL * g:DL * (g + 1)]).astype(BF),
            "wo": np.ascontiguousarray(W_o[DL * g:DL * (g + 1), :]).astype(BF),
        }
        im.update(masks)
        in_maps.append(im)
    res = run_bass_kernel_spmd(nc, in_maps, list(range(8)))
    global _last_res
    _last_res = res
    out = np.zeros((2, T, D), np.float32)
    for cc in range(8):
        out[cc // 4] += np.asarray(res.results[cc]["out"], dtype=np.float32)
    return out


# revision 10
# speedup vs baseline: 1.2533x; 1.0731x over previous
"""Trainium2 Bass kernel for nn_HLALayer (higher-order linear attention).

Math: the reference scan
    k_C = k_t @ C;  G += k_t k_t^T C;  S += k_t k_t^T;  C += q_t v_t^T
    o_t = q_t @ (S C - G)
admits a chunked closed form (chunk L), carrying only S and Z = S C - G:
    o_chunk = Q @ Z0 + tril(Q S0 Q^T + A tril(A)^T) @ V,   A = Q K^T
    S1 = S0 + K^T K
    Z1 = Z0 + S0 (Q^T V) + K^T (tril(K Q^T, incl diag) V)

v3: bf16 operands (PSUM accumulation stays f32) -> FWL weight loads,
halved DMA; host passes x pre-transposed (no PE transposes of x); trimmed
triangular matmuls; head-pair packing of 64-wide matmuls onto disjoint PE
row/col groups; single shared transpose PSUM + one cast per head.

Sharding: 8 cores = (batch b in {0,1}) x (head-group g in {0..3}, 4 heads
each).  Each core projects x[b] with its weight column/row slices, runs the
chunked scan for its 4 heads, applies its W_o row-slice -> partial [T, D]
bf16 output; the host sums the 4 partials per batch in f32.
"""

import numpy as np
import sys

sys.path.insert(0, "/opt/trn_rl_repo")

import ml_dtypes
import concourse.bacc as bacc
import concourse.mybir as mybir
from concourse.bass_utils import run_bass_kernel_spmd
from concourse.tile import TileContext

F32 = mybir.dt.float32
BF16 = mybir.dt.bfloat16
BF = ml_dtypes.bfloat16

D = 1024          # model dim
DL = 256          # per-core projection width (4 heads x 64)
DK = 64           # head dim
L = 256           # chunk length
NHL = 4           # local heads per core


def build(T=2048):
    NCH = T // L
    nc = bacc.Bacc("TRN2", target_bir_lowering=False)

    xt_in = nc.declare_dram_parameter("xt", [D, T], BF16, isOutput=False)
    wq_in = nc.declare_dram_parameter("wq", [D, DL], BF16, isOutput=False)
    wk_in = nc.declare_dram_parameter("wk", [D, DL], BF16, isOutput=False)
    wv_in = nc.declare_dram_parameter("wv", [D, DL], BF16, isOutput=False)
    wo_in = nc.declare_dram_parameter("wo", [DL, D], BF16, isOutput=False)
    id_in = nc.declare_dram_parameter("ident", [128, 128], BF16, isOutput=False)
    mtr_in = nc.declare_dram_parameter("mtr", [128, 128], F32, isOutput=False)  # triu
    mtrb_in = nc.declare_dram_parameter("mtrb", [128, 128], BF16, isOutput=False)
    mtl_in = nc.declare_dram_parameter("mtl", [128, 128], F32, isOutput=False)  # tril
    mt0_in = nc.declare_dram_parameter("mt0", [128, 256], F32, isOutput=False)  # [triu|1]
    out_d = nc.declare_dram_parameter("out", [T, D], BF16, isOutput=True)

    ncp = 0  # copy-engine round robin counter

    with TileContext(nc) as tc:
        with tc.tile_pool(name="const", bufs=1) as cpool, \
             tc.tile_pool(name="work", bufs=2) as work, \
             tc.tile_pool(name="spool", bufs=2) as spool, \
             tc.tile_pool(name="pp", bufs=2, space="PSUM") as pps:

            def cp(out_ap, in_ap):
                """plain copy/cast, alternating DVE / ACT to balance load"""
                nonlocal ncp
                ncp += 1
                if ncp % 2 == 0:
                    nc.vector.tensor_copy(out_ap, in_ap)
                else:
                    nc.scalar.copy(out_ap, in_ap)

            # ---- constants / weights (gpsimd=SWDGE queue, keeps HWDGE free) ----
            ident = cpool.tile([128, 128], BF16)
            nc.gpsimd.dma_start(out=ident[:], in_=id_in[:])
            mtr = cpool.tile([128, 128], F32)
            nc.gpsimd.dma_start(out=mtr[:], in_=mtr_in[:])
            mtrb = cpool.tile([128, 128], BF16)
            nc.gpsimd.dma_start(out=mtrb[:], in_=mtrb_in[:])
            mtl = cpool.tile([128, 128], F32)
            nc.gpsimd.dma_start(out=mtl[:], in_=mtl_in[:])
            mt0 = cpool.tile([128, 256], F32)
            nc.gpsimd.dma_start(out=mt0[:], in_=mt0_in[:])

            wq_sb, wk_sb, wv_sb = [], [], []
            for j in range(8):
                wqt = cpool.tile([128, DL], BF16, name=f"wq{j}")
                nc.gpsimd.dma_start(out=wqt[:], in_=wq_in[128 * j:128 * (j + 1), :])
                wq_sb.append(wqt)
                wkt = cpool.tile([128, DL], BF16, name=f"wk{j}")
                nc.gpsimd.dma_start(out=wkt[:], in_=wk_in[128 * j:128 * (j + 1), :])
                wk_sb.append(wkt)
                wvt = cpool.tile([128, DL], BF16, name=f"wv{j}")
                nc.gpsimd.dma_start(out=wvt[:], in_=wv_in[128 * j:128 * (j + 1), :])
                wv_sb.append(wvt)
            wo_sb = []
            for m in range(2):
                wot = cpool.tile([128, D], BF16, name=f"wo{m}")
                nc.gpsimd.dma_start(out=wot[:], in_=wo_in[128 * m:128 * (m + 1), :])
                wo_sb.append(wot)

            # per-pair states (h0 at partitions 0:64, h1 at 64:128), bf16
            S = [None] * 2
            Z = [None] * 2

            for c in range(NCH):
                t0 = L * c
                # ---------- phase 1: load x^T tiles, project ----------
                xt = []
                for j in range(8):
                    xtj = work.tile([128, L], BF16, tag=f"xt{j}", bufs=3,
                                    name=f"xt{j}_{c}")
                    nc.sync.dma_start(out=xtj[:],
                                      in_=xt_in[128 * j:128 * (j + 1), t0:t0 + L])
                    xt.append(xtj)

                qt, kt = [], []
                for m in range(2):
                    psq = pps.tile([128, L], F32, tag="pb", bufs=5, name=f"psq{m}_{c}")
                    for j in range(8):
                        nc.tensor.matmul(psq[:], wq_sb[j][:, 128 * m:128 * (m + 1)],
                                         xt[j][:], start=(j == 0), stop=(j == 7))
                    qtm = work.tile([128, L], BF16, tag=f"qt{m}", bufs=3,
                                    name=f"qt{m}_{c}")
                    cp(qtm[:], psq[:])
                    qt.append(qtm)
                    psk = pps.tile([128, L], F32, tag="pb", bufs=5, name=f"psk{m}_{c}")
                    for j in range(8):
                        nc.tensor.matmul(psk[:], wk_sb[j][:, 128 * m:128 * (m + 1)],
                                         xt[j][:], start=(j == 0), stop=(j == 7))
                    ktm = work.tile([128, L], BF16, tag=f"kt{m}", bufs=3,
                                    name=f"kt{m}_{c}")
                    cp(ktm[:], psk[:])
                    kt.append(ktm)
                vt = []
                for bb in range(2):
                    psv = pps.tile([128, DL], F32, tag="pb", bufs=5, name=f"psv{bb}_{c}")
                    for j in range(8):
                        nc.tensor.matmul(psv[:], xt[j][:, 128 * bb:128 * (bb + 1)],
                                         wv_sb[j][:], start=(j == 0), stop=(j == 7))
                    vtb = work.tile([128, DL], BF16, tag=f"vt{bb}", bufs=3,
                                    name=f"vt{bb}_{c}")
                    cp(vtb[:], psv[:])
                    vt.append(vtb)

                # per-chunk output tiles (oT layout [dv, t]; p=0: heads 0,1)
                ot = [work.tile([128, L], BF16, tag=f"ot{m}", bufs=3,
                                name=f"ot{m}_{c}") for m in range(2)]

                # ---------- phase 2: chunked scan, software-pipelined pairs ----
                PAIRS = []
                for p in range(2):
                    heads = (2 * p, 2 * p + 1)
                    st = {"p": p, "heads": heads, "QT": {}, "KT": {}, "Vbs": {},
                          "idb": {}}
                    for h in heads:
                        m, po = h >> 1, 64 * (h & 1)
                        st["QT"][h] = qt[m][po:po + 64, :]
                        st["KT"][h] = kt[m][po:po + 64, :]
                        st["Vbs"][h] = [vt[bb][:, 64 * h:64 * h + 64]
                                        for bb in range(2)]
                        st["idb"][h] = ident[po:po + 64, po:po + 64]
                    PAIRS.append(st)

                # ---- FRONT: NM/M products, transposes, masks, QST ----
                for st in PAIRS:
                    p, heads = st["p"], st["heads"]
                    QT, KT, idb = st["QT"], st["KT"], st["idb"]
                    # N: psNM = [N0 full (256) | N1 right (128)]
                    # M: psM  = [M0 left (128) | M1 full (256)]
                    psNM, psM = {}, {}
                    for h in heads:
                        psNM[h] = pps.tile([128, 384], F32, tag="pb", bufs=5,
                                           name=f"psNM_{c}_{h}")
                        nc.tensor.matmul(psNM[h][:, 0:256], KT[h][:, 0:128], QT[h],
                                         start=True, stop=True)
                        nc.tensor.matmul(psNM[h][:, 256:384], KT[h][:, 128:256],
                                         QT[h][:, 128:256], start=True, stop=True)
                    for h in heads:
                        psM[h] = pps.tile([128, 384], F32, tag="pb", bufs=5,
                                          name=f"psM_{c}_{h}")
                        nc.tensor.matmul(psM[h][:, 0:128], QT[h][:, 0:128],
                                         KT[h][:, 0:128], start=True, stop=True)
                        nc.tensor.matmul(psM[h][:, 128:384], QT[h][:, 128:256],
                                         KT[h][:], start=True, stop=True)

                    # natural q/k via PE transpose into one shared PSUM tile per
                    # head: qkn = [qn0 | qn1 | kn0 | kn1], one cast per head
                    qkn = {}
                    for h in heads:
                        psqk = pps.tile([128, 256], BF16, tag="ps", bufs=3,
                                        name=f"psqk_{c}_{h}")
                        for bb in range(2):
                            nc.tensor.transpose(psqk[:, 64 * bb:64 * (bb + 1)],
                                                QT[h][:, 128 * bb:128 * (bb + 1)],
                                                idb[h])
                            nc.tensor.transpose(psqk[:, 128 + 64 * bb:192 + 64 * bb],
                                                KT[h][:, 128 * bb:128 * (bb + 1)],
                                                idb[h])
                        qknh = work.tile([128, 256], BF16, tag=f"qkn{h}", bufs=2,
                                         name=f"qkn_{c}_{h}")
                        cp(qknh[:], psqk[:])
                        qkn[h] = qknh
                    st["qn"] = {h: [qkn[h][:, 0:64], qkn[h][:, 64:128]]
                                for h in heads}
                    st["kn"] = {h: [qkn[h][:, 128:192], qkn[h][:, 192:256]]
                                for h in heads}

                    # casts of N (PSUM frees after these two reads), then
                    # triangular masks on GpSimd from the SBUF copies
                    n0sb, n1sbR, triuN0L, triuN1 = {}, {}, {}, {}
                    trilM0L, m1L, trilM1R = {}, {}, {}
                    for h in heads:
                        n0sb[h] = work.tile([128, 256], BF16, tag=f"n0sb{h}", bufs=2,
                                            name=f"n0sb_{c}_{h}")
                        cp(n0sb[h][:], psNM[h][:, 0:256])
                        n1sbR[h] = work.tile([128, 128], BF16, tag=f"n1sbR{h}",
                                             bufs=2, name=f"n1sbR_{c}_{h}")
                        cp(n1sbR[h][:], psNM[h][:, 256:384])
                        triuN0L[h] = work.tile([128, 128], BF16, tag=f"tN0{h}",
                                               bufs=2, name=f"tN0_{c}_{h}")
                        nc.gpsimd.tensor_mul(triuN0L[h][:], n0sb[h][:, 0:128],
                                             mtrb[:])
                        triuN1[h] = work.tile([128, 128], BF16, tag=f"tN1{h}",
                                              bufs=2, name=f"tN1_{c}_{h}")
                        nc.gpsimd.tensor_mul(triuN1[h][:], n1sbR[h][:], mtrb[:])
                        trilM0L[h] = work.tile([128, 128], BF16, tag=f"lM0{h}",
                                               bufs=2, name=f"lM0_{c}_{h}")
                        nc.vector.tensor_mul(trilM0L[h][:], psM[h][:, 0:128], mtl[:])
                        m1L[h] = work.tile([128, 128], BF16, tag=f"m1L{h}", bufs=2,
                                           name=f"m1L_{c}_{h}")
                        cp(m1L[h][:], psM[h][:, 128:256])
                        trilM1R[h] = work.tile([128, 128], BF16, tag=f"lM1{h}",
                                               bufs=2, name=f"lM1_{c}_{h}")
                        nc.vector.tensor_mul(trilM1R[h][:], psM[h][:, 256:384],
                                             mtl[:])
                    st.update(n0sb=n0sb, n1sbR=n1sbR, triuN0L=triuN0L,
                              triuN1=triuN1, trilM0L=trilM0L, m1L=m1L,
                              trilM1R=trilM1R)

                    # QST = S0 @ QT per head, pair-packed [128, 256]
                    st["qstsb"] = None
                    if c > 0:
                        psQST = pps.tile([128, L], F32, tag="ps", bufs=3,
                                         name=f"psQST_{c}_{p}")
                        for h in heads:
                            po = 64 * (h & 1)
                            nc.tensor.matmul(psQST[po:po + 64, :],
                                             S[p][po:po + 64, :], QT[h],
                                             start=True, stop=True)
                        qstsb = work.tile([128, L], BF16, tag=f"qst{p}", bufs=2,
                                          name=f"qst_{c}_{p}")
                        cp(qstsb[:], psQST[:])
                        st["qstsb"] = qstsb

                # ---- MID: AT products + masks ----
                for st in PAIRS:
                    p, heads = st["p"], st["heads"]
                    QT, qstsb = st["QT"], st["qstsb"]
                    n0sb, n1sbR = st["n0sb"], st["n1sbR"]
                    triuN0L, triuN1 = st["triuN0L"], st["triuN1"]
                    psAT, at0, at1R = {}, {}, {}
                    for h in heads:
                        po = 64 * (h & 1)
                        psAT[h] = pps.tile([128, 384], F32, tag="pb", bufs=5,
                                           name=f"psAT_{c}_{h}")
                        nc.tensor.matmul(psAT[h][:, 0:256], triuN0L[h][:],
                                         n0sb[h][:], start=True, stop=(c == 0))
                        if c > 0:
                            nc.tensor.matmul(psAT[h][:, 0:256],
                                             qstsb[po:po + 64, 0:128], QT[h],
                                             start=False, stop=True)
                        nc.tensor.matmul(psAT[h][:, 256:384], n0sb[h][:, 128:256],
                                         n0sb[h][:, 128:256], start=True, stop=False)
                        nc.tensor.matmul(psAT[h][:, 256:384], triuN1[h][:],
                                         n1sbR[h][:], start=False, stop=(c == 0))
                        if c > 0:
                            nc.tensor.matmul(psAT[h][:, 256:384],
                                             qstsb[po:po + 64, 128:256],
                                             QT[h][:, 128:256],
                                             start=False, stop=True)
                    for h in heads:
                        at0[h] = work.tile([128, 256], BF16, tag=f"at0{h}", bufs=2,
                                           name=f"at0_{c}_{h}")
                        nc.vector.tensor_mul(at0[h][:], psAT[h][:, 0:256], mt0[:])
                        at1R[h] = work.tile([128, 128], BF16, tag=f"at1{h}", bufs=2,
                                            name=f"at1_{c}_{h}")
                        nc.vector.tensor_mul(at1R[h][:], psAT[h][:, 256:384], mtr[:])
                    st["at0"], st["at1R"] = at0, at1R

                # ---- BACK: outputs + state updates ----
                for st in PAIRS:
                    p, heads = st["p"], st["heads"]
                    QT, Vbs = st["QT"], st["Vbs"]
                    qn, kn = st["qn"], st["kn"]
                    at0, at1R = st["at0"], st["at1R"]
                    trilM0L, m1L, trilM1R = st["trilM0L"], st["m1L"], st["trilM1R"]

                    # oT = V^T tril(AT) + (Z0^T Q^T), pair-packed [128, 256]
                    psO = pps.tile([128, L], F32, tag="ps", bufs=3,
                                   name=f"psO_{c}_{p}")
                    for h in heads:
                        po = 64 * (h & 1)
                        nc.tensor.matmul(psO[po:po + 64, :], Vbs[h][0], at0[h][:],
                                         start=True, stop=False)
                        nc.tensor.matmul(psO[po:po + 64, 128:256], Vbs[h][1],
                                         at1R[h][:], start=False, stop=(c == 0))
                        if c > 0:
                            nc.tensor.matmul(psO[po:po + 64, :],
                                             Z[p][po:po + 64, :], QT[h],
                                             start=False, stop=True)
                    cp(ot[p][:], psO[:])

                    # dS = K^T K, dC = Q^T V (pair-packed [128, 64+64])
                    psSC = pps.tile([128, 128], F32, tag="ps", bufs=3,
                                    name=f"psSC_{c}_{p}")
                    for h in heads:
                        po = 64 * (h & 1)
                        nc.tensor.matmul(psSC[po:po + 64, 0:64], kn[h][0], kn[h][0],
                                         start=True, stop=False)
                        nc.tensor.matmul(psSC[po:po + 64, 0:64], kn[h][1], kn[h][1],
                                         start=False, stop=True)
                        if c > 0:
                            nc.tensor.matmul(psSC[po:po + 64, 64:128], qn[h][0],
                                             Vbs[h][0], start=True, stop=False)
                            nc.tensor.matmul(psSC[po:po + 64, 64:128], qn[h][1],
                                             Vbs[h][1], start=False, stop=True)

                    # W2 = tril(KQ^T, incl diag) V per head:
                    # psW = [W2(s 0:128) | W2(s 128:256)]
                    wsb = {}
                    for h in heads:
                        psW = pps.tile([128, 128], F32, tag="ps", bufs=3,
                                       name=f"psW_{c}_{h}")
                        nc.tensor.matmul(psW[:, 0:64], trilM0L[h][:], Vbs[h][0],
                                         start=True, stop=False)
                        nc.tensor.matmul(psW[:, 0:64], m1L[h][:], Vbs[h][1],
                                         start=False, stop=True)
                        nc.tensor.matmul(psW[:, 64:128], trilM1R[h][:], Vbs[h][1],
                                         start=True, stop=True)
                        wsb[h] = work.tile([128, 128], BF16, tag=f"wsb{h}", bufs=2,
                                           name=f"wsb_{c}_{h}")
                        cp(wsb[h][:], psW[:])

                    # dZ = K^T W2 (+ S0 dC), pair-packed
                    dcsb = None
                    if c > 0:
                        dcsb = work.tile([128, 64], BF16, tag=f"dcsb{p}", bufs=2,
                                         name=f"dcsb_{c}_{p}")
                        cp(dcsb[:], psSC[:, 64:128])
                    psZ = pps.tile([128, 64], F32, tag="ps", bufs=3,
                                   name=f"psZ_{c}_{p}")
                    for h in heads:
                        po = 64 * (h & 1)
                        nc.tensor.matmul(psZ[po:po + 64, :], kn[h][0],
                                         wsb[h][:, 0:64], start=True, stop=False)
                        nc.tensor.matmul(psZ[po:po + 64, :], kn[h][1],
                                         wsb[h][:, 64:128], start=False,
                                         stop=(c == 0))
                        if c > 0:
                            nc.tensor.matmul(psZ[po:po + 64, :], S[p][po:po + 64, :],
                                             dcsb[po:po + 64, :],
                                             start=False, stop=True)

                    # new states (one DVE op per state for the pair)
                    Snew = spool.tile([128, 64], BF16, tag=f"S{p}", bufs=2,
                                      name=f"S{p}_{c}")
                    Znew = spool.tile([128, 64], BF16, tag=f"Z{p}", bufs=2,
                                      name=f"Z{p}_{c}")
                    if c > 0:
                        nc.vector.tensor_add(Snew[:], S[p][:], psSC[:, 0:64])
                        nc.vector.tensor_add(Znew[:], Z[p][:], psZ[:])
                    else:
                        nc.vector.tensor_copy(Snew[:], psSC[:, 0:64])
                        nc.vector.tensor_copy(Znew[:], psZ[:])
                    S[p], Z[p] = Snew, Znew

                # ---------- phase 3: output projection for this chunk ----------
                for bb in range(2):
                    for ncol in range(2):
                        pso = pps.tile([128, 512], F32, tag="ps", bufs=3,
                                       name=f"pso{bb}{ncol}_{c}")
                        nc.tensor.matmul(pso[:], ot[0][:, 128 * bb:128 * (bb + 1)],
                                         wo_sb[0][:, 512 * ncol:512 * (ncol + 1)],
                                         start=True, stop=False)
                        nc.tensor.matmul(pso[:], ot[1][:, 128 * bb:128 * (bb + 1)],
                                         wo_sb[1][:, 512 * ncol:512 * (ncol + 1)],
                                         start=False, stop=True)
                        osb = work.tile([128, 512], BF16, tag="osb", bufs=3,
                                        name=f"osb{bb}{ncol}_{c}")
                        cp(osb[:], pso[:])
                        nc.scalar.dma_start(
                            out=out_d[t0 + 128 * bb:t0 + 128 * (bb + 1),
                                      512 * ncol:512 * (ncol + 1)],
                            in_=osb[:])

    nc.compile()
    return nc


def _masks():
    p = np.arange(128)[:, None]
    f = np.arange(128)[None, :]
    triu = (f >= p).astype(np.float32)
    tril = (f <= p).astype(np.float32)
    ones = np.ones((128, 128), np.float32)
    return {
        "ident": np.eye(128, dtype=BF),
        "mtr": triu,
        "mtrb": triu.astype(BF),
        "mtl": tril,
        "mt0": np.concatenate([triu, ones], axis=1),
    }


_NC_CACHE = {}


def get_nc(T=2048):
    if T not in _NC_CACHE:
        _NC_CACHE[T] = build(T)
    return _NC_CACHE[T]


def kernel(x, W_q, W_k, W_v, W_o):
    T = x.shape[1]
    nc = get_nc(T)
    masks = _masks()
    xts = [np.ascontiguousarray(x[b].T).astype(BF) for b in range(2)]
    in_maps = []
    for cc in range(8):
        b, g = cc // 4, cc % 4
        im = {
            "xt": xts[b],
            "wq": np.ascontiguousarray(W_q[:, DL * g:DL * (g + 1)]).astype(BF),
            "wk": np.ascontiguousarray(W_k[:, DL * g:DL * (g + 1)]).astype(BF),
            "wv": np.ascontiguousarray(W_v[:, DL * g:DL * (g + 1)]).astype(BF),
            "wo": np.ascontiguousarray(W_o[DL * g:DL * (g + 1), :]).astype(BF),
        }
        im.update(masks)
        in_maps.append(im)
    res = run_bass_kernel_spmd(nc, in_maps, list(range(8)))
    global _last_res
    _last_res = res
    out = np.zeros((2, T, D), np.float32)
    for cc in range(8):
        out[cc // 4] += np.asarray(res.results[cc]["out"], dtype=np.float32)
    return out
